# revision 29
# baseline (speedup 1.0000x reference)
"""Branching-Kriging pairwise kernel matrix on 8 Trainium2 NeuronCores.

Math: for rows i of W1 and j of W2,
    K(i,j) = exp(share_k + branch_k + nested_k)
Every term is a sum over products of a function of i and a function of j
(the categorical branch/level structure is one-hot encodable), so
    log K = F1 @ F2.T
with F1 [4096, D] and F2 [2048, D] feature matrices (D=96: 79 exact
columns + 17 fp16 residual-correction columns).

Per core (512 output rows): fp16 K=96 matmuls into PSUM fp32, exp on
ACT writing fp16 SBUF, fp16 output DMA (host upcasts to fp32).  The
input arrives in 5 pipelined pieces so the first matmul/exp starts
~1us after the first 120KB lands instead of after the full 480KB.
Output DMAs are fire-and-forget: their transfers complete during the
NEFF's fixed end-of-execution semaphore-sweep epilogue (~6.6us of
sequencer work that runs after the kernel body on every execution),
so no completion wait is on the critical path.
"""

import numpy as np

import concourse.bass as bass
import concourse.mybir as mybir
from concourse.bass_utils import run_bass_kernel_spmd

N_CORES = 8
N1, N2 = 4096, 2048
ROWS = N1 // N_CORES          # 512 output rows per core
D = 80                        # feature (contraction) dim: 79 + 1 correction
S, B = 8, 3                   # spatial / branching factor counts
NEST = [3, 3, 3]              # nested factors per branching factor

FP32 = mybir.dt.float32
FP16 = mybir.dt.float16


def _act(x):
    return np.minimum(np.where(x >= 0.0, x + 1.0, np.exp(x)), 30.0).astype(np.float32)


def _r16(x):
    return x.astype(np.float16).astype(np.float32)


def _build_features(W1, W2, alpha, theta, gamma0, gamma1, gamma2):
    """log K = F1 @ F2.T, exactly (up to fp16 rounding + corrections)."""
    W1 = np.asarray(W1, np.float32)
    W2 = np.asarray(W2, np.float32)
    n1, n2 = W1.shape[0], W2.shape[0]
    X1, Z1, V1 = W1[:, :S], W1[:, S:S + B], W1[:, S + B:]
    X2, Z2, V2 = W2[:, :S], W2[:, S:S + B], W2[:, S + B:]
    a = _act(np.asarray(alpha))[0]            # [S]
    t = _act(np.asarray(theta))[0]            # [B]
    G = [_act(np.asarray(g)) - 1.0 for g in (gamma0, gamma1, gamma2)]  # [nb, 4]

    F1 = np.zeros((n1, D), np.float32)
    F2 = np.zeros((n2, D), np.float32)

    # row terms + constant
    F1[:, 0] = 1.0
    F2[:, 0] = -(X2**2 @ a) - (V2**2).sum(1) - t.sum()
    F1[:, 1] = -(X1**2 @ a) - (V1**2).sum(1)
    F2[:, 1] = 1.0
    # share cross: 2 a_s x1 x2
    F1[:, 2:10] = 2.0 * a[None, :] * X1
    F2[:, 2:10] = X2
    # nested v cross (level-independent part): 2 v1 v2
    F1[:, 10:19] = 2.0 * V1
    F2[:, 10:19] = V2

    d = 19
    Z1i = Z1.astype(np.int32)
    Z2i = Z2.astype(np.int32)
    off = 0
    for b in range(B):
        nb = NEST[b]
        v1b = V1[:, off:off + nb]
        v2b = V2[:, off:off + nb]
        for lev in range(1, 5):
            e1 = (Z1i[:, b] == lev).astype(np.float32)
            e2 = (Z2i[:, b] == lev).astype(np.float32)
            g = G[b][:, lev - 1]
            # branch match reward t_b, minus gamma-weighted v2^2
            F1[:, d] = e1
            F2[:, d] = e2 * (t[b] - (v2b**2) @ g)
            d += 1
            # gamma-weighted v1^2
            F1[:, d] = -e1 * ((v1b**2) @ g)
            F2[:, d] = e2
            d += 1
            # gamma-weighted cross terms
            F1[:, d:d + nb] = 2.0 * e1[:, None] * v1b * g[None, :]
            F2[:, d:d + nb] = e2[:, None] * v2b
            d += nb
        off += nb
    assert d == 79

    # The matmul runs in fp16.  Pre-round both feature matrices to fp16 so
    # the hardware rounding is a no-op, then spend the spare contraction
    # dims (79..95) on residual-correction columns for the worst error
    # contributors: F*G = r(F)r(G) + L_F r(G) + r(F) L_G up to a negligible
    # L_F*L_G term.
    nd = d
    L1 = F1[:, :nd] - _r16(F1[:, :nd])
    L2 = F2[:, :nd] - _r16(F2[:, :nd])
    c1 = np.abs(L1).max(0) * np.abs(F2[:, :nd]).max(0)
    c2 = np.abs(F1[:, :nd]).max(0) * np.abs(L2).max(0)
    cand = [(c1[i], i, 1) for i in range(nd)] + [(c2[i], i, 2) for i in range(nd)]
    cand.sort(key=lambda t: -t[0])
    F1[:, :nd] = _r16(F1[:, :nd])
    F2[:, :nd] = _r16(F2[:, :nd])
    for c, i, side in cand[:D - nd]:
        if c <= 0.0:
            break
        if side == 1:
            F1[:, d] = _r16(L1[:, i])
            F2[:, d] = F2[:, i]
        else:
            F1[:, d] = F1[:, i]
            F2[:, d] = _r16(L2[:, i])
        d += 1
    return F1.astype(np.float16), F2.astype(np.float16)


_COMPILED = None

# fin column layout (per core):
#   [0:128)      w0   = F1shard.T block 0
#   [128:640)    f2c0 = F2.T cols 0:512
#   [640:1152)   f2c1 = F2.T cols 512:1024
#   [1152:1664)  f2c2 = F2.T cols 1024:1536
#   [1664:2176)  f2c3 = F2.T cols 1536:2048
#   [2176:2560)  w1|w2|w3 = F1shard.T blocks 1..3
# pieces sized so the first matmul's inputs (w0 + first 256 F2 cols)
# arrive as early as possible; later pieces stream in behind the
# compute pipeline
_PIECES = [(0, 384), (384, 1152), (1152, 1664), (1664, 2176), (2176, 2560)]


def _get_nc():
    """Raw Bass program (no TileContext), software-pipelined:

    Sync: 5 input DMAs (pieces, each gating the earliest matmul that
    needs it) then 6 output DMAs (no completion sems - transfers drain
    during the NEFF epilogue).  Tensor: 8 warm-up matmuls on garbage
    (spins up the PE DVFS clock during the input DMA), then 16 real
    512-col fp16 matmuls into two 4-bank PSUM tiles.  Scalar: ACT
    exp-table preload, then 6 exps (512/512/1024/2048x3) PSUM->SBUF
    fp16; the last output DMA is split between the Sync and Act HWDGE
    queues so its tail transfers in parallel.
    """
    global _COMPILED
    if _COMPILED is not None:
        return _COMPILED

    nc = bass.Bass(target_bir_lowering=False, debug=False)
    fin = nc.dram_tensor("fin", [D, 2560], FP16, kind="ExternalInput")
    out = nc.dram_tensor("out", [ROWS, N2], FP16, kind="ExternalOutput")

    MT = ROWS // 128          # 4 output row-blocks per core
    EXPF = mybir.ActivationFunctionType.Exp

    with (
        nc.sbuf_tensor("fins", [D, 2560], FP16) as fins,
        nc.sbuf_tensor("ots", [128, 8192], FP16) as ots,
        nc.sbuf_tensor("wbuf", [D, 512], FP16) as wbuf,
        nc.sbuf_tensor("scr", [128, 1], FP32) as scr,
        nc.psum_tensor("ps0", [128, N2], FP32) as ps0,
        nc.psum_tensor("ps1", [128, N2], FP32) as ps1,
        nc.semaphore("in0") as in0,
        nc.semaphore("in1") as in1,
        nc.semaphore("in2") as in2,
        nc.semaphore("in3") as in3,
        nc.semaphore("in4") as in4,
        nc.semaphore("mm_sem") as mm_sem,
        nc.semaphore("act_sem") as act_sem,
        nc.semaphore("out_sem") as out_sem,
        nc.Block() as block,
    ):
        ins = [in0, in1, in2, in3, in4]
        pss = [ps0, ps1]

        def w(mt):       # stationary [D, 128] block for row-block mt
            if mt == 0:
                return fins[:, 0:128]
            return fins[:, 2176 + (mt - 1) * 128:2176 + mt * 128]

        def f2c(c):      # moving [D, 512] chunk c of F2.T
            return fins[:, 128 + c * 512:128 + (c + 1) * 512]

        @block.sync
        def _(sync):
            for k, (lo, hi) in enumerate(_PIECES):
                sync.dma_start(fins[:, lo:hi], fin[:, lo:hi]).then_inc(ins[k], 16)
            # out_sem is incremented (codegen requires sync info on DGE)
            # but never waited on: the transfers drain during the NEFF
            # epilogue, which also resets the semaphores.
            outs = [
                (1, out[0:128, 0:256], ots[:, 0:256]),
                (2, out[0:128, 256:512], ots[:, 256:512]),
                (3, out[0:128, 512:1024], ots[:, 512:1024]),
                (4, out[0:128, 1024:2048], ots[:, 1024:2048]),
                (5, out[128:256, :], ots[:, 2048:4096]),
                (6, out[256:384, :], ots[:, 4096:6144]),
                (7, out[384:512, 0:1536], ots[:, 6144:7680]),
            ]
            for k, dst, src in outs:
                sync.wait_ge(act_sem, k)
                sync.dma_start(dst, src).then_inc(out_sem, 16)

        @block.tensor
        def _(tensor):
            tensor.wait_ge(in0, 16)
            nc.tensor.matmul(ps0[:, 0:256], w(0), fins[:, 128:384],
                             start=True, stop=True).then_inc(mm_sem)
            tensor.wait_ge(in1, 16)
            nc.tensor.matmul(ps0[:, 256:512], w(0), fins[:, 384:640],
                             start=True, stop=True).then_inc(mm_sem)
            nc.tensor.matmul(ps0[:, 512:1024], w(0), f2c(1),
                             start=True, stop=True).then_inc(mm_sem)
            tensor.wait_ge(in2, 16)
            nc.tensor.matmul(ps0[:, 1024:1536], w(0), f2c(2),
                             start=True, stop=True).then_inc(mm_sem)
            tensor.wait_ge(in3, 16)
            nc.tensor.matmul(ps0[:, 1536:2048], w(0), f2c(3),
                             start=True, stop=True).then_inc(mm_sem)
            tensor.wait_ge(in4, 16)
            for mt in range(1, MT):
                ps = pss[mt % 2]
                if mt == 2:
                    # ps0 reuse: wait until its four exps have read it out
                    tensor.wait_ge(act_sem, 4)
                if mt == 3:
                    # ps1 reuse: wait for its exp
                    tensor.wait_ge(act_sem, 5)
                for c in range(4):
                    nc.tensor.matmul(ps[:, c * 512:(c + 1) * 512], w(mt), f2c(c),
                                     start=True, stop=True).then_inc(mm_sem)

        @block.scalar
        def _(scalar):
            # dummy 1-column activation so the ACT table load is hoisted to
            # kernel start (overlapping the input DMA) instead of stalling
            # the first real exp by ~1.3us
            one = nc.const_aps.aps[(mybir.dt.float32, 1.0)]
            nc.scalar.activation(scr[:], one, EXPF)
            tiles = [
                (1, ps0[:, 0:256], 0, 256),
                (2, ps0[:, 256:512], 256, 256),
                (3, ps0[:, 512:1024], 512, 512),
                (5, ps0[:, 1024:2048], 1024, 1024),
                (9, ps1[:, 0:2048], 2048, 2048),
                (13, ps0[:, 0:2048], 4096, 2048),
                (16, ps1[:, 0:1536], 6144, 1536),
                (17, ps1[:, 1536:2048], 7680, 512),
            ]
            for mmk, src, o, n in tiles:
                scalar.wait_ge(mm_sem, mmk)
                nc.scalar.activation(ots[:, o:o + n], src, EXPF).then_inc(act_sem)
            # last row-block's 512-col tail rides the Act HWDGE queue so it
            # transfers in parallel with the Sync queue's 1536-col piece.
            # wait_ge on our own act_sem: the HWDGE descriptor fetch is NOT
            # ordered after the preceding exp's completion, so without this
            # the transfer can read ots before the exp wrote it.
            scalar.wait_ge(act_sem, 8)
            nc.scalar.dma_start(out[384:512, 1536:2048],
                                ots[:, 7680:8192]).then_inc(out_sem, 16)

    _COMPILED = nc
    return _COMPILED


LAST_RESULTS = None


def _ensure_ntff_hook():
    """The agent image's `antenv` lacks `axon_hooks`; register the
    boot-shipped ctypes NTFF hook under that name so trace=True works."""
    import sys
    import types

    try:
        import antenv.axon_hooks  # noqa: F401
        return
    except ImportError:
        pass
    mod = types.ModuleType("antenv.axon_hooks")
    mod._hook = None

    def set_axon_ntff_profile_hook(hook):
        mod._hook = hook

    def get_axon_ntff_profile_hook():
        return mod._hook

    mod.set_axon_ntff_profile_hook = set_axon_ntff_profile_hook
    mod.get_axon_ntff_profile_hook = get_axon_ntff_profile_hook
    sys.modules["antenv.axon_hooks"] = mod
    import antenv

    antenv.axon_hooks = mod
    try:
        from trn_agent_boot.trn_boot import _ntff_profile_via_ctypes

        mod._hook = _ntff_profile_via_ctypes("/opt/axon/libaxon_pjrt.so")
    except Exception:
        pass
    # artifact upload needs bucket creds this container may not have;
    # the local NTFF -> perfetto pipeline doesn't depend on it
    import concourse.bass_utils as _bu

    _orig_upload = _bu.upload_artifacts

    def _safe_upload(tmpdir):
        try:
            return _orig_upload(tmpdir)
        except Exception:
            return tmpdir

    _bu.upload_artifacts = _safe_upload


def kernel(W1, W2, alpha, theta, gamma0, gamma1, gamma2, _profile=False):
    global LAST_RESULTS
    if _profile:
        _ensure_ntff_hook()
    F1, F2 = _build_features(W1, W2, alpha, theta, gamma0, gamma1, gamma2)
    f1t = np.ascontiguousarray(F1.T)      # [D, N1] fp16
    f2t = np.ascontiguousarray(F2.T)      # [D, N2] fp16
    in_maps = []
    for c in range(N_CORES):
        blk = f1t[:, c * ROWS:(c + 1) * ROWS]      # [D, 512]
        fin = np.concatenate(
            [blk[:, 0:128], f2t, blk[:, 128:512]], axis=1
        )
        in_maps.append({"fin": np.ascontiguousarray(fin)})
    nc = _get_nc()
    res = run_bass_kernel_spmd(nc, in_maps, list(range(N_CORES)), trace=_profile)
    LAST_RESULTS = res
    return np.concatenate(
        [res.results[c]["out"] for c in range(N_CORES)], axis=0
    ).astype(np.float32)


# revision 30
# speedup vs baseline: 1.1395x; 1.1395x over previous
"""Branching-Kriging pairwise kernel matrix on 8 Trainium2 NeuronCores.

Math: for rows i of W1 and j of W2,
    K(i,j) = exp(share_k + branch_k + nested_k)
Every term is a sum over products of a function of i and a function of j
(the categorical branch/level structure is one-hot encodable), so
    log K = F1 @ F2.T
with F1 [4096, D] and F2 [2048, D] feature matrices (D=80: 79 exact
columns + 1 fp16 residual-correction column).

Per core (512 output rows): fp16 K=80 matmuls into PSUM fp32, exp on
ACT writing fp16 SBUF, fp16 output DMA (host upcasts to fp32).  The
input arrives in 5 pipelined pieces so the first matmul/exp starts as
soon as the first 60KB lands instead of after the full 400KB, and the
exp tiles are sized fine early (256/256/512) and coarse late (2048) to
start the ACT chain at the earliest possible moment without paying
per-instruction overhead in steady state.  Output DMAs are never
waited on: their transfers complete during the NEFF's fixed
end-of-execution semaphore-sweep epilogue (~6.6us of sequencer work
that runs after the kernel body on every execution), so no completion
wait is on the critical path.
"""

import numpy as np

import concourse.bass as bass
import concourse.mybir as mybir
from concourse.bass_utils import run_bass_kernel_spmd

N_CORES = 8
N1, N2 = 4096, 2048
ROWS = N1 // N_CORES          # 512 output rows per core
D = 80                        # feature (contraction) dim: 79 + 1 correction
S, B = 8, 3                   # spatial / branching factor counts
NEST = [3, 3, 3]              # nested factors per branching factor

FP32 = mybir.dt.float32
FP16 = mybir.dt.float16


def _act(x):
    return np.minimum(np.where(x >= 0.0, x + 1.0, np.exp(x)), 30.0).astype(np.float32)


def _r16(x):
    return x.astype(np.float16).astype(np.float32)


def _build_features(W1, W2, alpha, theta, gamma0, gamma1, gamma2):
    """log K = F1 @ F2.T, exactly (up to fp16 rounding + corrections)."""
    W1 = np.asarray(W1, np.float32)
    W2 = np.asarray(W2, np.float32)
    n1, n2 = W1.shape[0], W2.shape[0]
    X1, Z1, V1 = W1[:, :S], W1[:, S:S + B], W1[:, S + B:]
    X2, Z2, V2 = W2[:, :S], W2[:, S:S + B], W2[:, S + B:]
    a = _act(np.asarray(alpha))[0]            # [S]
    t = _act(np.asarray(theta))[0]            # [B]
    G = [_act(np.asarray(g)) - 1.0 for g in (gamma0, gamma1, gamma2)]  # [nb, 4]

    F1 = np.zeros((n1, D), np.float32)
    F2 = np.zeros((n2, D), np.float32)

    # row terms + constant
    F1[:, 0] = 1.0
    F2[:, 0] = -(X2**2 @ a) - (V2**2).sum(1) - t.sum()
    F1[:, 1] = -(X1**2 @ a) - (V1**2).sum(1)
    F2[:, 1] = 1.0
    # share cross: 2 a_s x1 x2
    F1[:, 2:10] = 2.0 * a[None, :] * X1
    F2[:, 2:10] = X2
    # nested v cross (level-independent part): 2 v1 v2
    F1[:, 10:19] = 2.0 * V1
    F2[:, 10:19] = V2

    d = 19
    Z1i = Z1.astype(np.int32)
    Z2i = Z2.astype(np.int32)
    off = 0
    for b in range(B):
        nb = NEST[b]
        v1b = V1[:, off:off + nb]
        v2b = V2[:, off:off + nb]
        for lev in range(1, 5):
            e1 = (Z1i[:, b] == lev).astype(np.float32)
            e2 = (Z2i[:, b] == lev).astype(np.float32)
            g = G[b][:, lev - 1]
            # branch match reward t_b, minus gamma-weighted v2^2
            F1[:, d] = e1
            F2[:, d] = e2 * (t[b] - (v2b**2) @ g)
            d += 1
            # gamma-weighted v1^2
            F1[:, d] = -e1 * ((v1b**2) @ g)
            F2[:, d] = e2
            d += 1
            # gamma-weighted cross terms
            F1[:, d:d + nb] = 2.0 * e1[:, None] * v1b * g[None, :]
            F2[:, d:d + nb] = e2[:, None] * v2b
            d += nb
        off += nb
    assert d == 79

    # The matmul runs in fp16.  Pre-round both feature matrices to fp16 so
    # the hardware rounding is a no-op, then spend the spare contraction
    # dims (79..95) on residual-correction columns for the worst error
    # contributors: F*G = r(F)r(G) + L_F r(G) + r(F) L_G up to a negligible
    # L_F*L_G term.
    nd = d
    L1 = F1[:, :nd] - _r16(F1[:, :nd])
    L2 = F2[:, :nd] - _r16(F2[:, :nd])
    c1 = np.abs(L1).max(0) * np.abs(F2[:, :nd]).max(0)
    c2 = np.abs(F1[:, :nd]).max(0) * np.abs(L2).max(0)
    cand = [(c1[i], i, 1) for i in range(nd)] + [(c2[i], i, 2) for i in range(nd)]
    cand.sort(key=lambda t: -t[0])
    F1[:, :nd] = _r16(F1[:, :nd])
    F2[:, :nd] = _r16(F2[:, :nd])
    for c, i, side in cand[:D - nd]:
        if c <= 0.0:
            break
        if side == 1:
            F1[:, d] = _r16(L1[:, i])
            F2[:, d] = F2[:, i]
        else:
            F1[:, d] = F1[:, i]
            F2[:, d] = _r16(L2[:, i])
        d += 1
    return F1.astype(np.float16), F2.astype(np.float16)


_COMPILED = None

# fin column layout (per core):
#   [0:128)      w0   = F1shard.T block 0
#   [128:640)    f2c0 = F2.T cols 0:512
#   [640:1152)   f2c1 = F2.T cols 512:1024
#   [1152:1664)  f2c2 = F2.T cols 1024:1536
#   [1664:2176)  f2c3 = F2.T cols 1536:2048
#   [2176:2560)  w1|w2|w3 = F1shard.T blocks 1..3
# pieces sized so the first matmul's inputs (w0 + first 256 F2 cols)
# arrive as early as possible; later pieces stream in behind the
# compute pipeline
_PIECES = [(0, 384), (384, 1152), (1152, 1664), (1664, 2176), (2176, 2560)]


def _get_nc():
    """Raw Bass program (no TileContext), software-pipelined:

    Sync: 5 input DMAs (pieces, each gating the earliest matmul that
    needs it) then 7 output DMAs whose completion is never waited on
    (transfers drain during the NEFF epilogue).  Tensor: 17 fp16
    matmuls (256/256/512x14) into two 4-bank PSUM tiles.  Scalar: ACT
    exp-table preload, then 8 exps (256/256/512/1024/2048x3/1536/512)
    PSUM->SBUF fp16; the last row-block's output is split between the
    Sync and Act HWDGE queues so the two tails transfer in parallel.
    """
    global _COMPILED
    if _COMPILED is not None:
        return _COMPILED

    nc = bass.Bass(target_bir_lowering=False, debug=False)
    fin = nc.dram_tensor("fin", [D, 2560], FP16, kind="ExternalInput")
    out = nc.dram_tensor("out", [ROWS, N2], FP16, kind="ExternalOutput")

    MT = ROWS // 128          # 4 output row-blocks per core
    EXPF = mybir.ActivationFunctionType.Exp

    with (
        nc.sbuf_tensor("fins", [D, 2560], FP16) as fins,
        nc.sbuf_tensor("ots", [128, 8192], FP16) as ots,
        nc.sbuf_tensor("scr", [128, 1], FP32) as scr,
        nc.psum_tensor("ps0", [128, N2], FP32) as ps0,
        nc.psum_tensor("ps1", [128, N2], FP32) as ps1,
        nc.semaphore("in0") as in0,
        nc.semaphore("in1") as in1,
        nc.semaphore("in2") as in2,
        nc.semaphore("in3") as in3,
        nc.semaphore("in4") as in4,
        nc.semaphore("mm_sem") as mm_sem,
        nc.semaphore("act_sem") as act_sem,
        nc.semaphore("out_sem") as out_sem,
        nc.Block() as block,
    ):
        ins = [in0, in1, in2, in3, in4]
        pss = [ps0, ps1]

        def w(mt):       # stationary [D, 128] block for row-block mt
            if mt == 0:
                return fins[:, 0:128]
            return fins[:, 2176 + (mt - 1) * 128:2176 + mt * 128]

        def f2c(c):      # moving [D, 512] chunk c of F2.T
            return fins[:, 128 + c * 512:128 + (c + 1) * 512]

        @block.sync
        def _(sync):
            for k, (lo, hi) in enumerate(_PIECES):
                sync.dma_start(fins[:, lo:hi], fin[:, lo:hi]).then_inc(ins[k], 16)
            # out_sem is incremented (codegen requires sync info on DGE)
            # but never waited on: the transfers drain during the NEFF
            # epilogue, which also resets the semaphores.
            outs = [
                (1, out[0:128, 0:256], ots[:, 0:256]),
                (2, out[0:128, 256:512], ots[:, 256:512]),
                (3, out[0:128, 512:1024], ots[:, 512:1024]),
                (4, out[0:128, 1024:2048], ots[:, 1024:2048]),
                (5, out[128:256, :], ots[:, 2048:4096]),
                (6, out[256:384, :], ots[:, 4096:6144]),
                (7, out[384:512, 0:1536], ots[:, 6144:7680]),
            ]
            for k, dst, src in outs:
                sync.wait_ge(act_sem, k)
                sync.dma_start(dst, src).then_inc(out_sem, 16)

        @block.tensor
        def _(tensor):
            tensor.wait_ge(in0, 16)
            nc.tensor.matmul(ps0[:, 0:256], w(0), fins[:, 128:384],
                             start=True, stop=True).then_inc(mm_sem)
            tensor.wait_ge(in1, 16)
            nc.tensor.matmul(ps0[:, 256:512], w(0), fins[:, 384:640],
                             start=True, stop=True).then_inc(mm_sem)
            nc.tensor.matmul(ps0[:, 512:1024], w(0), f2c(1),
                             start=True, stop=True).then_inc(mm_sem)
            tensor.wait_ge(in2, 16)
            nc.tensor.matmul(ps0[:, 1024:1536], w(0), f2c(2),
                             start=True, stop=True).then_inc(mm_sem)
            tensor.wait_ge(in3, 16)
            nc.tensor.matmul(ps0[:, 1536:2048], w(0), f2c(3),
                             start=True, stop=True).then_inc(mm_sem)
            tensor.wait_ge(in4, 16)
            for mt in range(1, MT):
                ps = pss[mt % 2]
                if mt == 2:
                    # ps0 reuse: wait until its four exps have read it out
                    tensor.wait_ge(act_sem, 4)
                if mt == 3:
                    # ps1 reuse: wait for its exp
                    tensor.wait_ge(act_sem, 5)
                for c in range(4):
                    nc.tensor.matmul(ps[:, c * 512:(c + 1) * 512], w(mt), f2c(c),
                                     start=True, stop=True).then_inc(mm_sem)

        @block.scalar
        def _(scalar):
            # dummy 1-column activation so the ACT table load is hoisted to
            # kernel start (overlapping the input DMA) instead of stalling
            # the first real exp by ~1.3us
            one = nc.const_aps.aps[(mybir.dt.float32, 1.0)]
            nc.scalar.activation(scr[:], one, EXPF)
            tiles = [
                (1, ps0[:, 0:256], 0, 256),
                (2, ps0[:, 256:512], 256, 256),
                (3, ps0[:, 512:1024], 512, 512),
                (5, ps0[:, 1024:2048], 1024, 1024),
                (9, ps1[:, 0:2048], 2048, 2048),
                (13, ps0[:, 0:2048], 4096, 2048),
                (16, ps1[:, 0:1536], 6144, 1536),
                (17, ps1[:, 1536:2048], 7680, 512),
            ]
            for mmk, src, o, n in tiles:
                scalar.wait_ge(mm_sem, mmk)
                nc.scalar.activation(ots[:, o:o + n], src, EXPF).then_inc(act_sem)
            # last row-block's 512-col tail rides the Act HWDGE queue so it
            # transfers in parallel with the Sync queue's 1536-col piece.
            # wait_ge on our own act_sem: the HWDGE descriptor fetch is NOT
            # ordered after the preceding exp's completion, so without this
            # the transfer can read ots before the exp wrote it.
            scalar.wait_ge(act_sem, 8)
            nc.scalar.dma_start(out[384:512, 1536:2048],
                                ots[:, 7680:8192]).then_inc(out_sem, 16)

    _COMPILED = nc
    return _COMPILED


LAST_RESULTS = None


def _ensure_ntff_hook():
    """The agent image's `antenv` lacks `axon_hooks`; register the
    boot-shipped ctypes NTFF hook under that name so trace=True works."""
    import sys
    import types

    try:
        import antenv.axon_hooks  # noqa: F401
        return
    except ImportError:
        pass
    mod = types.ModuleType("antenv.axon_hooks")
    mod._hook = None

    def set_axon_ntff_profile_hook(hook):
        mod._hook = hook

    def get_axon_ntff_profile_hook():
        return mod._hook

    mod.set_axon_ntff_profile_hook = set_axon_ntff_profile_hook
    mod.get_axon_ntff_profile_hook = get_axon_ntff_profile_hook
    sys.modules["antenv.axon_hooks"] = mod
    import antenv

    antenv.axon_hooks = mod
    try:
        from trn_agent_boot.trn_boot import _ntff_profile_via_ctypes

        mod._hook = _ntff_profile_via_ctypes("/opt/axon/libaxon_pjrt.so")
    except Exception:
        pass
    # artifact upload needs bucket creds this container may not have;
    # the local NTFF -> perfetto pipeline doesn't depend on it
    import concourse.bass_utils as _bu

    _orig_upload = _bu.upload_artifacts

    def _safe_upload(tmpdir):
        try:
            return _orig_upload(tmpdir)
        except Exception:
            return tmpdir

    _bu.upload_artifacts = _safe_upload


def kernel(W1, W2, alpha, theta, gamma0, gamma1, gamma2, _profile=False):
    global LAST_RESULTS
    if _profile:
        _ensure_ntff_hook()
    F1, F2 = _build_features(W1, W2, alpha, theta, gamma0, gamma1, gamma2)
    f1t = np.ascontiguousarray(F1.T)      # [D, N1] fp16
    f2t = np.ascontiguousarray(F2.T)      # [D, N2] fp16
    in_maps = []
    for c in range(N_CORES):
        blk = f1t[:, c * ROWS:(c + 1) * ROWS]      # [D, 512]
        fin = np.concatenate(
            [blk[:, 0:128], f2t, blk[:, 128:512]], axis=1
        )
        in_maps.append({"fin": np.ascontiguousarray(fin)})
    nc = _get_nc()
    res = run_bass_kernel_spmd(nc, in_maps, list(range(N_CORES)), trace=_profile)
    LAST_RESULTS = res
    return np.concatenate(
        [res.results[c]["out"] for c in range(N_CORES)], axis=0
    ).astype(np.float32)
